# revision 27
# baseline (speedup 1.0000x reference)
"""AttentionPool Trainium2 kernel.

Problem: x[B=8, S=4096, D=768] f32; att_v[768]; att_W[768, 768].
  y = tanh(x @ W); scores = y . v; w = softmax(scores over S); out = w . x  -> [B, D]

Sharding: pure data-parallel over batch B — one batch per NeuronCore, 8 cores,
no collectives.

Per-core pipeline (batch b), per 128-row sequence tile i:
  1. HWDGE f32 load of x tile into a staging ring (full-rate, no cast)
  2. PE transpose-mode (matmul vs identity, f32): x_tile -> xT psum
  3. DVE copy-cast psum f32 -> SBUF bf16 xT
  4. PE: y = xT.T @ W (bf16, psum f32; 6 k-chunks x {512, 256})
  5. ACT: t = tanh(y_psum) -> f32
  6. DVE: scores_i = sum_e t*v   (scalar_tensor_tensor accum)
  7. ACT: u_i = exp(scores_i)  (no max-subtraction needed: |scores| < ~0.5),
     accum_out -> Z partial column
  8. PE: p += u_i.T @ x_stage_i  (f32 pooling, accumulated in PSUM)
Host: out = p / Z  (Z = sum of the per-partition exp accums).

Emission is software-pipelined (transposes of tile i are emitted before the
consume-chain of tile i-1) so the PE never waits on the single-buffered
transpose PSUM bank.
"""

import sys

sys.path.insert(0, "/opt/trn_rl_repo")

import numpy as np

import concourse.bass as bass
import concourse.mybir as mybir
import concourse.tile as tile
from concourse.bass_utils import run_bass_kernel_spmd
from concourse.masks import make_identity

P = 128
S = 4096
D = 768
NT = S // P  # 32 sequence tiles
DJ = D // P  # 6 contraction chunks
NCORES = 8

F32 = mybir.dt.float32
BF16 = mybir.dt.bfloat16
ACTF = mybir.ActivationFunctionType


def _build(split_waits: bool = True) -> bass.Bass:
    nc = bass.Bass()
    x_d = nc.declare_dram_parameter("x", [S, D], F32, isOutput=False)
    v_d = nc.declare_dram_parameter("att_v", [D], F32, isOutput=False)
    w_d = nc.declare_dram_parameter("att_W", [D, D], F32, isOutput=False)
    p_d = nc.declare_dram_parameter("out_p", [4, D], F32, isOutput=True)
    z_d = nc.declare_dram_parameter("out_z", [P, NT], F32, isOutput=True)

    with tile.TileContext(nc) as tc:
        with (
            tc.tile_pool(name="singles", bufs=1) as singles,
            tc.tile_pool(name="stage", bufs=6) as stage_pool,
            tc.tile_pool(name="xt", bufs=4) as xt_pool,
            tc.tile_pool(name="tbuf", bufs=3) as t_pool,
            tc.tile_pool(name="sc", bufs=6) as sc_pool,
            tc.tile_pool(name="ypsum", bufs=2, space="PSUM") as ypsum_pool,
            tc.tile_pool(name="xtpsum", bufs=1, space="PSUM") as xtp_pool,
            tc.tile_pool(name="ppsum", bufs=1, space="PSUM") as ppsum_pool,
        ):
            # v broadcast along partitions, f32 (STT runs 1x either way)
            v_bc = singles.tile([P, D], F32)
            nc.sync.dma_start(out=v_bc, in_=v_d[:][None, :].to_broadcast([P, D]))
            # identity for PE transpose-mode (bf16: f32 matmuls stream the
            # moving operand at half rate, so transposes run bf16)
            ident = singles.tile([P, P], BF16)
            make_identity(nc, ident)
            # W: f32 load (HWDGE, parallel lanes), bf16 conversion per-chunk
            # on the otherwise-idle GpSimd so the first y-matmuls aren't
            # gated on the full 2.25 MB.
            w_f32 = singles.tile([P, DJ, D], F32)
            w_sb = singles.tile([P, DJ, D], BF16)
            for j in range(DJ):
                nc.sync.dma_start(
                    out=w_f32[:, j, :], in_=w_d[j * P : (j + 1) * P, :]
                )
                nc.gpsimd.tensor_copy(out=w_sb[:, j, :], in_=w_f32[:, j, :])
            # per-tile partial Z accumulators; host sums the 128*NT values.
            zg = singles.tile([P, NT], F32)
            # pooling accumulator psum: 4 col-group accumulator rows
            # (partitions 0/32/64/96), summed on the host
            p_ps = ppsum_pool.tile([P, D], F32)

            stage_tiles = {}
            xt_tiles = {}

            def emit_front(i):
                # load + bf16 cast + transpose + copy for tile i
                xs = stage_pool.tile([P, D], F32, name="xs")
                nc.sync.dma_start(out=xs, in_=x_d[i * P : (i + 1) * P, :])
                stage_tiles[i] = xs
                xb = stage_pool.tile([P, D], BF16, name="xb")
                nc.gpsimd.tensor_copy(out=xb, in_=xs)
                xt_ps = xtp_pool.tile([P, D], BF16, name="xt_ps")
                for j in range(DJ):
                    nc.tensor.transpose(
                        xt_ps[:, j * P : (j + 1) * P],
                        xb[:, j * P : (j + 1) * P],
                        ident,
                    )
                xt = xt_pool.tile([P, D], BF16, name="xt")
                nc.vector.tensor_copy(out=xt, in_=xt_ps)
                xt_tiles[i] = xt

            def emit_back(i):
                # y matmuls + tanh + scores + exp + pooling for tile i
                xt = xt_tiles.pop(i)
                yps = ypsum_pool.tile([P, D], F32, name="yps")
                for j in range(DJ):
                    nc.tensor.matmul(
                        yps[:, 0:512],
                        lhsT=xt[:, j * P : (j + 1) * P],
                        rhs=w_sb[:, j, 0:512],
                        start=(j == 0),
                        stop=(j == DJ - 1),
                    )
                    nc.tensor.matmul(
                        yps[:, 512:D],
                        lhsT=xt[:, j * P : (j + 1) * P],
                        rhs=w_sb[:, j, 512:D],
                        start=(j == 0),
                        stop=(j == DJ - 1),
                    )
                t = t_pool.tile([P, D], F32, name="t")
                nc.scalar.activation(out=t, in_=yps, func=ACTF.Tanh)
                scores = sc_pool.tile([P, 1], F32, name="scores")
                dve_out = t_pool.tile([P, D], F32, name="dve_out")
                nc.vector.scalar_tensor_tensor(
                    out=dve_out,
                    in0=t,
                    scalar=1.0,
                    in1=v_bc,
                    op0=mybir.AluOpType.mult,
                    op1=mybir.AluOpType.mult,
                    accum_out=scores,
                )
                u = sc_pool.tile([P, 1], F32, name="u")
                nc.scalar.activation(
                    out=u, in_=scores, func=ACTF.Exp,
                    accum_out=zg[:, i : i + 1],
                )
                # pooling: 4 independent accumulator rows in col-groups
                # 0/32/64/96 — consecutive tiles' M=1 matmuls run
                # concurrently in distinct 32-column strips of the PE array
                xs = stage_tiles.pop(i)
                base = 32 * (i % 4)
                nc.tensor.matmul(
                    p_ps[base : base + 1, 0:512], lhsT=u, rhs=xs[:, 0:512],
                    start=(i < 4), stop=(i >= NT - 4),
                    tile_position=(0, base),
                    skip_group_check=True,
                )
                nc.tensor.matmul(
                    p_ps[base : base + 1, 512:D], lhsT=u, rhs=xs[:, 512:D],
                    start=(i < 4), stop=(i >= NT - 4),
                    tile_position=(0, base),
                    skip_group_check=True,
                )

            for i in range(NT + 1):
                if i < NT:
                    emit_front(i)
                if i >= 1:
                    emit_back(i - 1)

            # write out unnormalized p rows and the Z partials; the host sums
            # rows 0/32/64/96 of out_p and all of out_z
            p_sb = singles.tile([P, D], F32)
            for base in (0, 32, 64, 96):
                nc.scalar.copy(
                    out=p_sb[base : base + 1, :], in_=p_ps[base : base + 1, :]
                )
            nc.sync.dma_start(out=p_d[:, :], in_=p_sb[0:97:32, :])
            nc.sync.dma_start(out=z_d[:, :], in_=zg)

    if split_waits:
        _split_excess_waits(nc)
    return nc


def _split_excess_waits(nc: bass.Bass) -> None:
    """Walrus accepts a single HW sync-wait per instruction (EventSemaphore
    excepted). Tile can attach more (data dep + DMA-lane reuse). Move all but
    one wait onto InstEventSemaphore(s) inserted just before, on the same
    engine — the sequencer executes waits in order, so semantics are
    unchanged."""
    fn = nc.m.functions[0]
    for blk in fn.blocks:
        insts = blk.instructions
        new_insts = []
        for inst in insts:
            si = inst.sync_info
            if (
                not isinstance(inst, mybir.InstEventSemaphore)
                and si is not None
                and len(si.on_wait) > 1
            ):
                waits = list(si.on_wait)
                for w in waits[:-1]:
                    ev = mybir.InstEventSemaphore(
                        name=nc.get_next_instruction_name(), ins=[], outs=[]
                    )
                    ev.engine = inst.engine
                    ev.sync_info = mybir.SyncInfo(on_wait=[w], on_update=[])
                    new_insts.append(ev)
                inst.sync_info = mybir.SyncInfo(
                    on_wait=waits[-1:], on_update=list(si.on_update)
                )
            new_insts.append(inst)
        blk.instructions = new_insts


_CACHE: dict = {}
LAST_RESULT = None


def _get_nc() -> bass.Bass:
    if "nc" not in _CACHE:
        _CACHE["nc"] = _build()
    return _CACHE["nc"]


def kernel(x: np.ndarray, att_v: np.ndarray, att_W: np.ndarray) -> np.ndarray:
    global LAST_RESULT
    assert x.shape == (NCORES, S, D), x.shape
    nc = _get_nc()
    in_maps = [
        {
            "x": np.ascontiguousarray(x[b], dtype=np.float32),
            "att_v": np.ascontiguousarray(att_v, dtype=np.float32),
            "att_W": np.ascontiguousarray(att_W, dtype=np.float32),
        }
        for b in range(NCORES)
    ]
    res = run_bass_kernel_spmd(nc, in_maps, core_ids=list(range(NCORES)))
    LAST_RESULT = res
    outs = []
    for b in range(NCORES):
        p = res.results[b]["out_p"].sum(axis=0, dtype=np.float64)
        z = res.results[b]["out_z"].sum(dtype=np.float64)
        outs.append(p / z)
    return np.stack(outs).astype(np.float32)


# revision 32
# speedup vs baseline: 1.2154x; 1.2154x over previous
"""AttentionPool Trainium2 kernel.

Problem: x[B=8, S=4096, D=768] f32; att_v[768]; att_W[768, 768].
  y = tanh(x @ W); scores = y . v; w = softmax(scores over S); out = w . x  -> [B, D]

Sharding: pure data-parallel over batch B — one batch per NeuronCore, 8 cores,
no collectives.

Per-core pipeline (batch b), per 128-row sequence tile i:
  1. HWDGE f32 load of x tile into a staging ring (full-rate, no cast)
  2. PE transpose-mode (matmul vs identity, f32): x_tile -> xT psum
  3. DVE copy-cast psum f32 -> SBUF bf16 xT
  4. PE: y = xT.T @ W (bf16, psum f32; 6 k-chunks x {512, 256})
  5. ACT: t = tanh(y_psum) -> f32
  6. DVE: scores_i = sum_e t*v   (scalar_tensor_tensor accum)
  7. ACT: u_i = exp(scores_i)  (no max-subtraction needed: |scores| < ~0.5),
     accum_out -> Z partial column
  8. PE: p += u_i.T @ x_stage_i  (f32 pooling, accumulated in PSUM)
Host: out = p / Z  (Z = sum of the per-partition exp accums).

Emission is software-pipelined (transposes of tile i are emitted before the
consume-chain of tile i-1) so the PE never waits on the single-buffered
transpose PSUM bank.
"""

import sys

sys.path.insert(0, "/opt/trn_rl_repo")

import numpy as np

import concourse.bass as bass
import concourse.mybir as mybir
import concourse.tile as tile
from concourse.bass_utils import run_bass_kernel_spmd
from concourse.masks import make_identity

P = 128
S = 4096
D = 768
NT = S // P  # 32 sequence tiles
DJ = D // P  # 6 contraction chunks
NCORES = 8

F32 = mybir.dt.float32
BF16 = mybir.dt.bfloat16
ACTF = mybir.ActivationFunctionType


def _build(split_waits: bool = True) -> bass.Bass:
    nc = bass.Bass()
    x_d = nc.declare_dram_parameter("x", [S, D], F32, isOutput=False)
    v_d = nc.declare_dram_parameter("att_v", [D], F32, isOutput=False)
    w_d = nc.declare_dram_parameter("att_W", [D, D], F32, isOutput=False)
    p_d = nc.declare_dram_parameter("out_p", [4, D], F32, isOutput=True)
    z_d = nc.declare_dram_parameter("out_z", [P, NT], F32, isOutput=True)

    with tile.TileContext(nc) as tc:
        with (
            tc.tile_pool(name="singles", bufs=1) as singles,
            tc.tile_pool(name="stage", bufs=9) as stage_pool,
            tc.tile_pool(name="xt", bufs=4) as xt_pool,
            tc.tile_pool(name="tbuf", bufs=3) as t_pool,
            tc.tile_pool(name="sc", bufs=9) as sc_pool,
            tc.tile_pool(name="ypsum", bufs=2, space="PSUM") as ypsum_pool,
            tc.tile_pool(name="xtpsum", bufs=1, space="PSUM") as xtp_pool,
            tc.tile_pool(name="ppsum", bufs=1, space="PSUM") as ppsum_pool,
        ):
            # v broadcast along partitions, f32 (STT runs 1x either way)
            v_bc = singles.tile([P, D], F32)
            nc.sync.dma_start(out=v_bc, in_=v_d[:][None, :].to_broadcast([P, D]))
            # identity for PE transpose-mode (bf16: f32 matmuls stream the
            # moving operand at half rate, so transposes run bf16)
            ident = singles.tile([P, P], BF16)
            make_identity(nc, ident)
            # W: f32 load (HWDGE, parallel lanes), bf16 conversion per-chunk
            # on the otherwise-idle GpSimd so the first y-matmuls aren't
            # gated on the full 2.25 MB.
            w_f32 = singles.tile([P, DJ, D], F32)
            w_sb = singles.tile([P, DJ, D], BF16)
            for j in range(DJ):
                nc.sync.dma_start(
                    out=w_f32[:, j, :], in_=w_d[j * P : (j + 1) * P, :]
                )
                nc.scalar.copy(out=w_sb[:, j, :], in_=w_f32[:, j, :])
            # per-tile partial Z accumulators; host sums the 128*NT values.
            zg = singles.tile([P, NT], F32)
            # pooling accumulator psum: 4 col-group accumulator rows
            # (partitions 0/32/64/96), summed on the host
            p_ps = ppsum_pool.tile([P, D], F32)

            stage_tiles = {}
            xt_tiles = {}
            u_tiles = {}

            def emit_front(i):
                # load + bf16 cast + transpose + copy for tile i
                xs = stage_pool.tile([P, D], F32, name="xs")
                nc.sync.dma_start(out=xs, in_=x_d[i * P : (i + 1) * P, :])
                stage_tiles[i] = xs
                xb = stage_pool.tile([P, D], BF16, name="xb")
                # alternate the cast between ACT and DVE to balance load
                if i % 2 == 0:
                    nc.scalar.copy(out=xb, in_=xs)
                else:
                    nc.vector.tensor_copy(out=xb, in_=xs)
                xt_ps = xtp_pool.tile([P, D], BF16, name="xt_ps")
                for j in range(DJ):
                    nc.tensor.transpose(
                        xt_ps[:, j * P : (j + 1) * P],
                        xb[:, j * P : (j + 1) * P],
                        ident,
                    )
                xt = xt_pool.tile([P, D], BF16, name="xt")
                nc.vector.tensor_copy(out=xt, in_=xt_ps)
                xt_tiles[i] = xt

            def emit_back(i):
                # y matmuls + tanh + scores + exp + pooling for tile i
                xt = xt_tiles.pop(i)
                yps = ypsum_pool.tile([P, D], F32, name="yps")
                for j in range(DJ):
                    nc.tensor.matmul(
                        yps[:, 0:512],
                        lhsT=xt[:, j * P : (j + 1) * P],
                        rhs=w_sb[:, j, 0:512],
                        start=(j == 0),
                        stop=(j == DJ - 1),
                    )
                    nc.tensor.matmul(
                        yps[:, 512:D],
                        lhsT=xt[:, j * P : (j + 1) * P],
                        rhs=w_sb[:, j, 512:D],
                        start=(j == 0),
                        stop=(j == DJ - 1),
                    )
                t = t_pool.tile([P, D], F32, name="t")
                nc.scalar.activation(out=t, in_=yps, func=ACTF.Tanh)
                scores = sc_pool.tile([P, 1], F32, name="scores")
                dve_out = t_pool.tile([P, D], F32, name="dve_out")
                nc.vector.scalar_tensor_tensor(
                    out=dve_out,
                    in0=t,
                    scalar=1.0,
                    in1=v_bc,
                    op0=mybir.AluOpType.mult,
                    op1=mybir.AluOpType.mult,
                    accum_out=scores,
                )
                u = sc_pool.tile([P, 1], F32, name="u")
                nc.scalar.activation(
                    out=u, in_=scores, func=ACTF.Exp,
                    accum_out=zg[:, i : i + 1],
                )
                u_tiles[i] = u

            def emit_pool_group(k):
                # pooling for tiles k-3..k: 8 adjacent M=1 matmuls into 4
                # independent accumulator rows (col-groups 0/32/64/96) so
                # they run concurrently in distinct 32-column strips of the
                # PE array. Deferred 3 tiles behind the main chain so the PE
                # never stalls waiting for exp.
                for i in range(k - 3, k + 1):
                    u = u_tiles.pop(i)
                    xs = stage_tiles.pop(i)
                    base = 32 * (i % 4)
                    nc.tensor.matmul(
                        p_ps[base : base + 1, 0:512], lhsT=u, rhs=xs[:, 0:512],
                        start=(i < 4), stop=(i >= NT - 4),
                        tile_position=(0, base),
                        skip_group_check=True,
                    )
                    nc.tensor.matmul(
                        p_ps[base : base + 1, 512:D], lhsT=u, rhs=xs[:, 512:D],
                        start=(i < 4), stop=(i >= NT - 4),
                        tile_position=(0, base),
                        skip_group_check=True,
                    )

            for i in range(NT + 3):
                if i < NT:
                    emit_front(i)
                if 1 <= i <= NT:
                    emit_back(i - 1)
                k = i - 3
                if k >= 3 and k % 4 == 3:
                    emit_pool_group(k)

            # write out unnormalized p rows and the Z partials; the host sums
            # rows 0/32/64/96 of out_p and all of out_z
            p_sb = singles.tile([P, D], F32)
            for base in (0, 32, 64, 96):
                nc.scalar.copy(
                    out=p_sb[base : base + 1, :], in_=p_ps[base : base + 1, :]
                )
            nc.sync.dma_start(out=p_d[:, :], in_=p_sb[0:97:32, :])
            nc.sync.dma_start(out=z_d[:, :], in_=zg)

    if split_waits:
        _split_excess_waits(nc)
    return nc


def _split_excess_waits(nc: bass.Bass) -> None:
    """Walrus accepts a single HW sync-wait per instruction (EventSemaphore
    excepted). Tile can attach more (data dep + DMA-lane reuse). Move all but
    one wait onto InstEventSemaphore(s) inserted just before, on the same
    engine — the sequencer executes waits in order, so semantics are
    unchanged."""
    fn = nc.m.functions[0]
    for blk in fn.blocks:
        insts = blk.instructions
        new_insts = []
        for inst in insts:
            si = inst.sync_info
            if (
                not isinstance(inst, mybir.InstEventSemaphore)
                and si is not None
                and len(si.on_wait) > 1
            ):
                waits = list(si.on_wait)
                for w in waits[:-1]:
                    ev = mybir.InstEventSemaphore(
                        name=nc.get_next_instruction_name(), ins=[], outs=[]
                    )
                    ev.engine = inst.engine
                    ev.sync_info = mybir.SyncInfo(on_wait=[w], on_update=[])
                    new_insts.append(ev)
                inst.sync_info = mybir.SyncInfo(
                    on_wait=waits[-1:], on_update=list(si.on_update)
                )
            new_insts.append(inst)
        blk.instructions = new_insts


_CACHE: dict = {}
LAST_RESULT = None


def _get_nc() -> bass.Bass:
    if "nc" not in _CACHE:
        _CACHE["nc"] = _build()
    return _CACHE["nc"]


def kernel(x: np.ndarray, att_v: np.ndarray, att_W: np.ndarray) -> np.ndarray:
    global LAST_RESULT
    assert x.shape == (NCORES, S, D), x.shape
    nc = _get_nc()
    in_maps = [
        {
            "x": np.ascontiguousarray(x[b], dtype=np.float32),
            "att_v": np.ascontiguousarray(att_v, dtype=np.float32),
            "att_W": np.ascontiguousarray(att_W, dtype=np.float32),
        }
        for b in range(NCORES)
    ]
    res = run_bass_kernel_spmd(nc, in_maps, core_ids=list(range(NCORES)))
    LAST_RESULT = res
    outs = []
    for b in range(NCORES):
        p = res.results[b]["out_p"].sum(axis=0, dtype=np.float64)
        z = res.results[b]["out_z"].sum(dtype=np.float64)
        outs.append(p / z)
    return np.stack(outs).astype(np.float32)


# revision 33
# speedup vs baseline: 1.4678x; 1.2077x over previous
"""AttentionPool Trainium2 kernel.

Problem: x[B=8, S=4096, D=768] f32; att_v[768]; att_W[768, 768].
  y = tanh(x @ W); scores = y . v; w = softmax(scores over S); out = w . x  -> [B, D]

Sharding: pure data-parallel over batch B — one batch per NeuronCore, 8 cores,
no collectives.

Per-core pipeline (batch b), per 128-row sequence tile i:
  1. HWDGE f32 load of x tile into a staging ring (full-rate, no cast)
  2. PE transpose-mode (matmul vs identity, f32): x_tile -> xT psum
  3. DVE copy-cast psum f32 -> SBUF bf16 xT
  4. PE: y = xT.T @ W (bf16, psum f32; 6 k-chunks x {512, 256})
  5. ACT: t = tanh(y_psum) -> f32
  6. DVE: scores_i = sum_e t*v   (scalar_tensor_tensor accum)
  7. ACT: u_i = exp(scores_i)  (no max-subtraction needed: |scores| < ~0.5),
     accum_out -> Z partial column
  8. PE: p += u_i.T @ x_stage_i  (f32 pooling, accumulated in PSUM)
Host: out = p / Z  (Z = sum of the per-partition exp accums).

Emission is software-pipelined (transposes of tile i are emitted before the
consume-chain of tile i-1) so the PE never waits on the single-buffered
transpose PSUM bank.
"""

import sys

sys.path.insert(0, "/opt/trn_rl_repo")

import numpy as np

import concourse.bass as bass
import concourse.mybir as mybir
import concourse.tile as tile
from concourse.bass_utils import run_bass_kernel_spmd
from concourse.masks import make_identity

P = 128
S = 4096
D = 768
NT = S // P  # 32 sequence tiles
DJ = D // P  # 6 contraction chunks
NCORES = 8

F32 = mybir.dt.float32
BF16 = mybir.dt.bfloat16
ACTF = mybir.ActivationFunctionType


def _build(split_waits: bool = True) -> bass.Bass:
    nc = bass.Bass()
    x_d = nc.declare_dram_parameter("x", [S, D], F32, isOutput=False)
    v_d = nc.declare_dram_parameter("att_v", [D], F32, isOutput=False)
    w_d = nc.declare_dram_parameter("att_W", [D, D], F32, isOutput=False)
    p_d = nc.declare_dram_parameter("out_p", [4, D], F32, isOutput=True)
    z_d = nc.declare_dram_parameter("out_z", [P, NT // 4], F32, isOutput=True)

    with tile.TileContext(nc) as tc:
        with (
            tc.tile_pool(name="singles", bufs=1) as singles,
            tc.tile_pool(name="stage", bufs=9) as stage_pool,
            tc.tile_pool(name="xt", bufs=4) as xt_pool,
            tc.tile_pool(name="tbuf", bufs=3) as t_pool,
            tc.tile_pool(name="sc", bufs=9) as sc_pool,
            tc.tile_pool(name="ypsum", bufs=2, space="PSUM") as ypsum_pool,
            tc.tile_pool(name="xtpsum", bufs=1, space="PSUM") as xtp_pool,
            tc.tile_pool(name="ppsum", bufs=1, space="PSUM") as ppsum_pool,
        ):
            # v broadcast along partitions; bf16 with a bf16 t gives the
            # STT a shot at the DVE 2x (16-bit) mode
            v_f32 = singles.tile([P, D], F32)
            nc.sync.dma_start(out=v_f32, in_=v_d[:][None, :].to_broadcast([P, D]))
            v_bc = singles.tile([P, D], BF16)
            nc.vector.tensor_copy(out=v_bc, in_=v_f32)
            # identity for PE transpose-mode (f32, matching the x tiles; the
            # copy out of PSUM folds the bf16 cast)
            ident = singles.tile([P, P], F32)
            make_identity(nc, ident)
            # W: f32 load (HWDGE, parallel lanes), bf16 conversion per-chunk
            # on the otherwise-idle GpSimd so the first y-matmuls aren't
            # gated on the full 2.25 MB.
            w_f32 = singles.tile([P, DJ, D], F32)
            w_sb = singles.tile([P, DJ, D], BF16)
            for j in range(DJ):
                nc.sync.dma_start(
                    out=w_f32[:, j, :], in_=w_d[j * P : (j + 1) * P, :]
                )
                nc.scalar.copy(out=w_sb[:, j, :], in_=w_f32[:, j, :])
            # per-group partial Z accumulators; host sums the values.
            zg = singles.tile([P, NT // 4], F32)
            # pooling accumulator psum: 4 col-group accumulator rows
            # (partitions 0/32/64/96), summed on the host
            p_ps = ppsum_pool.tile([P, D], F32)

            stage_tiles = {}
            xt_tiles = {}
            u_tiles = {}
            sc_tiles = {}

            def emit_front(i):
                # load + bf16 cast + transpose + copy for tile i
                xs = stage_pool.tile([P, D], F32, name="xs")
                nc.sync.dma_start(out=xs, in_=x_d[i * P : (i + 1) * P, :])
                stage_tiles[i] = xs
                xt_ps = xtp_pool.tile([P, D], F32, name="xt_ps")
                for j in range(DJ):
                    nc.tensor.transpose(
                        xt_ps[:, j * P : (j + 1) * P],
                        xs[:, j * P : (j + 1) * P],
                        ident,
                    )
                xt = xt_pool.tile([P, D], BF16, name="xt")
                nc.vector.tensor_copy(out=xt, in_=xt_ps)
                xt_tiles[i] = xt

            def emit_back(i):
                # y matmuls + tanh + scores + exp + pooling for tile i
                xt = xt_tiles.pop(i)
                yps = ypsum_pool.tile([P, D], F32, name="yps")
                for j in range(DJ):
                    nc.tensor.matmul(
                        yps[:, 0:512],
                        lhsT=xt[:, j * P : (j + 1) * P],
                        rhs=w_sb[:, j, 0:512],
                        start=(j == 0),
                        stop=(j == DJ - 1),
                    )
                    nc.tensor.matmul(
                        yps[:, 512:D],
                        lhsT=xt[:, j * P : (j + 1) * P],
                        rhs=w_sb[:, j, 512:D],
                        start=(j == 0),
                        stop=(j == DJ - 1),
                    )
                t = t_pool.tile([P, D], BF16, name="t")
                nc.scalar.activation(out=t, in_=yps, func=ACTF.Tanh)
                if i % 4 == 0:
                    sc4 = sc_pool.tile([P, 4], F32, name="sc4")
                    sc_tiles[i // 4] = sc4
                sc4 = sc_tiles[i // 4]
                dve_out = t_pool.tile([P, D], BF16, name="dve_out")
                nc.vector.scalar_tensor_tensor(
                    out=dve_out,
                    in0=t,
                    scalar=1.0,
                    in1=v_bc,
                    op0=mybir.AluOpType.mult,
                    op1=mybir.AluOpType.mult,
                    accum_out=sc4[:, i % 4 : i % 4 + 1],
                )
                if i % 4 == 3:
                    g = i // 4
                    u4 = sc_pool.tile([P, 4], F32, name="u4")
                    nc.scalar.activation(
                        out=u4, in_=sc_tiles.pop(g), func=ACTF.Exp,
                        accum_out=zg[:, g : g + 1],
                    )
                    u_tiles[g] = u4

            def emit_pool_group(k):
                # pooling for tiles k-3..k: 8 adjacent M=1 matmuls into 4
                # independent accumulator rows (col-groups 0/32/64/96) so
                # they run concurrently in distinct 32-column strips of the
                # PE array. Deferred 3 tiles behind the main chain so the PE
                # never stalls waiting for exp.
                u4 = u_tiles.pop(k // 4)
                for i in range(k - 3, k + 1):
                    u = u4[:, i % 4 : i % 4 + 1]
                    xs = stage_tiles.pop(i)
                    base = 32 * (i % 4)
                    nc.tensor.matmul(
                        p_ps[base : base + 1, 0:512], lhsT=u, rhs=xs[:, 0:512],
                        start=(i < 4), stop=(i >= NT - 4),
                        tile_position=(0, base),
                        skip_group_check=True,
                    )
                    nc.tensor.matmul(
                        p_ps[base : base + 1, 512:D], lhsT=u, rhs=xs[:, 512:D],
                        start=(i < 4), stop=(i >= NT - 4),
                        tile_position=(0, base),
                        skip_group_check=True,
                    )

            for i in range(NT + 3):
                if i < NT:
                    emit_front(i)
                if 1 <= i <= NT:
                    emit_back(i - 1)
                k = i - 3
                if k >= 3 and k % 4 == 3:
                    emit_pool_group(k)

            # write out unnormalized p rows and the Z partials; the host sums
            # rows 0/32/64/96 of out_p and all of out_z
            p_sb = singles.tile([P, D], F32)
            for base in (0, 32, 64, 96):
                nc.scalar.copy(
                    out=p_sb[base : base + 1, :], in_=p_ps[base : base + 1, :]
                )
            nc.sync.dma_start(out=p_d[:, :], in_=p_sb[0:97:32, :])
            nc.sync.dma_start(out=z_d[:, :], in_=zg)

    if split_waits:
        _split_excess_waits(nc)
    return nc


def _split_excess_waits(nc: bass.Bass) -> None:
    """Walrus accepts a single HW sync-wait per instruction (EventSemaphore
    excepted). Tile can attach more (data dep + DMA-lane reuse). Move all but
    one wait onto InstEventSemaphore(s) inserted just before, on the same
    engine — the sequencer executes waits in order, so semantics are
    unchanged."""
    fn = nc.m.functions[0]
    for blk in fn.blocks:
        insts = blk.instructions
        new_insts = []
        for inst in insts:
            si = inst.sync_info
            if (
                not isinstance(inst, mybir.InstEventSemaphore)
                and si is not None
                and len(si.on_wait) > 1
            ):
                waits = list(si.on_wait)
                for w in waits[:-1]:
                    ev = mybir.InstEventSemaphore(
                        name=nc.get_next_instruction_name(), ins=[], outs=[]
                    )
                    ev.engine = inst.engine
                    ev.sync_info = mybir.SyncInfo(on_wait=[w], on_update=[])
                    new_insts.append(ev)
                inst.sync_info = mybir.SyncInfo(
                    on_wait=waits[-1:], on_update=list(si.on_update)
                )
            new_insts.append(inst)
        blk.instructions = new_insts


_CACHE: dict = {}
LAST_RESULT = None


def _get_nc() -> bass.Bass:
    if "nc" not in _CACHE:
        _CACHE["nc"] = _build()
    return _CACHE["nc"]


def kernel(x: np.ndarray, att_v: np.ndarray, att_W: np.ndarray) -> np.ndarray:
    global LAST_RESULT
    assert x.shape == (NCORES, S, D), x.shape
    nc = _get_nc()
    in_maps = [
        {
            "x": np.ascontiguousarray(x[b], dtype=np.float32),
            "att_v": np.ascontiguousarray(att_v, dtype=np.float32),
            "att_W": np.ascontiguousarray(att_W, dtype=np.float32),
        }
        for b in range(NCORES)
    ]
    res = run_bass_kernel_spmd(nc, in_maps, core_ids=list(range(NCORES)))
    LAST_RESULT = res
    outs = []
    for b in range(NCORES):
        p = res.results[b]["out_p"].sum(axis=0, dtype=np.float64)
        z = res.results[b]["out_z"].sum(dtype=np.float64)
        outs.append(p / z)
    return np.stack(outs).astype(np.float32)


# revision 35
# speedup vs baseline: 1.5135x; 1.0311x over previous
"""AttentionPool Trainium2 kernel.

Problem: x[B=8, S=4096, D=768] f32; att_v[768]; att_W[768, 768].
  y = tanh(x @ W); scores = y . v; w = softmax(scores over S); out = w . x  -> [B, D]

Sharding: pure data-parallel over batch B — one batch per NeuronCore, 8 cores,
no collectives.

Per-core pipeline (batch b), per 128-row sequence tile i:
  1. HWDGE f32 load of x tile into a staging ring (full-rate, no cast)
  2. PE transpose-mode (matmul vs identity, f32): x_tile -> xT psum
  3. DVE copy-cast psum f32 -> SBUF bf16 xT
  4. PE: y = xT.T @ W (bf16, psum f32; 6 k-chunks x {512, 256})
  5. ACT: t = tanh(y_psum) -> f32
  6. DVE: scores_i = sum_e t*v   (scalar_tensor_tensor accum)
  7. ACT: u_i = exp(scores_i)  (no max-subtraction needed: |scores| < ~0.5),
     accum_out -> Z partial column
  8. PE: p += u_i.T @ x_stage_i  (f32 pooling, accumulated in PSUM)
Host: out = p / Z  (Z = sum of the per-partition exp accums).

Emission is software-pipelined (transposes of tile i are emitted before the
consume-chain of tile i-1) so the PE never waits on the single-buffered
transpose PSUM bank.
"""

import sys

sys.path.insert(0, "/opt/trn_rl_repo")

import numpy as np

import concourse.bass as bass
import concourse.mybir as mybir
import concourse.tile as tile
from concourse.bass_utils import run_bass_kernel_spmd
from concourse.masks import make_identity

P = 128
S = 4096
D = 768
NT = S // P  # 32 sequence tiles
DJ = D // P  # 6 contraction chunks
NCORES = 8

F32 = mybir.dt.float32
BF16 = mybir.dt.bfloat16
ACTF = mybir.ActivationFunctionType


def _build(split_waits: bool = True) -> bass.Bass:
    nc = bass.Bass()
    x_d = nc.declare_dram_parameter("x", [S, D], F32, isOutput=False)
    v_d = nc.declare_dram_parameter("att_v", [D], F32, isOutput=False)
    w_d = nc.declare_dram_parameter("att_W", [D, D], F32, isOutput=False)
    p_d = nc.declare_dram_parameter("out_p", [4, D], F32, isOutput=True)
    z_d = nc.declare_dram_parameter("out_z", [P, NT // 4], F32, isOutput=True)

    with tile.TileContext(nc) as tc:
        with (
            tc.tile_pool(name="singles", bufs=1) as singles,
            tc.tile_pool(name="stage", bufs=9) as stage_pool,
            tc.tile_pool(name="xt", bufs=4) as xt_pool,
            tc.tile_pool(name="tbuf", bufs=3) as t_pool,
            tc.tile_pool(name="sc", bufs=9) as sc_pool,
            tc.tile_pool(name="ypsum", bufs=2, space="PSUM") as ypsum_pool,
            tc.tile_pool(name="xtpsum", bufs=1, space="PSUM") as xtp_pool,
            tc.tile_pool(name="ppsum", bufs=1, space="PSUM") as ppsum_pool,
        ):
            # identity for PE transpose-mode (f32, matching the x tiles; the
            # copy out of PSUM folds the bf16 cast). GpSimd ops — no DMA ring
            # involvement, ready before the first x tile lands.
            ident = singles.tile([P, P], F32)
            make_identity(nc, ident)
            v_f32 = singles.tile([P, D], F32)
            v_bc = singles.tile([P, D], BF16)
            w_f32 = singles.tile([P, DJ, D], F32)
            w_sb = singles.tile([P, DJ, D], BF16)
            # per-group partial Z accumulators; host sums the values.
            zg = singles.tile([P, NT // 4], F32)
            # pooling accumulator psum: 4 col-group accumulator rows
            # (partitions 0/32/64/96), summed on the host
            p_ps = ppsum_pool.tile([P, D], F32)

            stage_tiles = {}
            xt_tiles = {}
            u_tiles = {}
            sc_tiles = {}

            def emit_params():
                # Emitted after the first two x loads: the HWDGE ring drains
                # in order, and the 128-descriptor v broadcast + 2.25 MB of W
                # must not gate the first transposes.
                for j in range(DJ):
                    nc.sync.dma_start(
                        out=w_f32[:, j, :], in_=w_d[j * P : (j + 1) * P, :]
                    )
                    nc.scalar.copy(out=w_sb[:, j, :], in_=w_f32[:, j, :])
                nc.sync.dma_start(
                    out=v_f32, in_=v_d[:][None, :].to_broadcast([P, D])
                )
                nc.vector.tensor_copy(out=v_bc, in_=v_f32)

            def emit_front(i):
                # load + bf16 cast + transpose + copy for tile i
                xs = stage_pool.tile([P, D], F32, name="xs")
                nc.sync.dma_start(out=xs, in_=x_d[i * P : (i + 1) * P, :])
                stage_tiles[i] = xs
                xt_ps = xtp_pool.tile([P, D], F32, name="xt_ps")
                for j in range(DJ):
                    nc.tensor.transpose(
                        xt_ps[:, j * P : (j + 1) * P],
                        xs[:, j * P : (j + 1) * P],
                        ident,
                    )
                xt = xt_pool.tile([P, D], BF16, name="xt")
                nc.vector.tensor_copy(out=xt, in_=xt_ps)
                xt_tiles[i] = xt

            def emit_back(i):
                # y matmuls + tanh + scores + exp + pooling for tile i
                xt = xt_tiles.pop(i)
                yps = ypsum_pool.tile([P, D], F32, name="yps")
                for j in range(DJ):
                    nc.tensor.matmul(
                        yps[:, 0:512],
                        lhsT=xt[:, j * P : (j + 1) * P],
                        rhs=w_sb[:, j, 0:512],
                        start=(j == 0),
                        stop=(j == DJ - 1),
                    )
                    nc.tensor.matmul(
                        yps[:, 512:D],
                        lhsT=xt[:, j * P : (j + 1) * P],
                        rhs=w_sb[:, j, 512:D],
                        start=(j == 0),
                        stop=(j == DJ - 1),
                    )
                t = t_pool.tile([P, D], BF16, name="t")
                nc.scalar.activation(out=t, in_=yps, func=ACTF.Tanh)
                if i % 4 == 0:
                    sc4 = sc_pool.tile([P, 4], F32, name="sc4")
                    sc_tiles[i // 4] = sc4
                sc4 = sc_tiles[i // 4]
                dve_out = t_pool.tile([P, D], BF16, name="dve_out")
                nc.vector.scalar_tensor_tensor(
                    out=dve_out,
                    in0=t,
                    scalar=1.0,
                    in1=v_bc,
                    op0=mybir.AluOpType.mult,
                    op1=mybir.AluOpType.mult,
                    accum_out=sc4[:, i % 4 : i % 4 + 1],
                )
                if i % 4 == 3:
                    g = i // 4
                    u4 = sc_pool.tile([P, 4], F32, name="u4")
                    nc.scalar.activation(
                        out=u4, in_=sc_tiles.pop(g), func=ACTF.Exp,
                        accum_out=zg[:, g : g + 1],
                    )
                    u_tiles[g] = u4

            def emit_pool_group(k):
                # pooling for tiles k-3..k: 8 adjacent M=1 matmuls into 4
                # independent accumulator rows (col-groups 0/32/64/96) so
                # they run concurrently in distinct 32-column strips of the
                # PE array. Deferred 3 tiles behind the main chain so the PE
                # never stalls waiting for exp.
                u4 = u_tiles.pop(k // 4)
                for i in range(k - 3, k + 1):
                    u = u4[:, i % 4 : i % 4 + 1]
                    xs = stage_tiles.pop(i)
                    base = 32 * (i % 4)
                    nc.tensor.matmul(
                        p_ps[base : base + 1, 0:512], lhsT=u, rhs=xs[:, 0:512],
                        start=(i < 4), stop=(i >= NT - 4),
                        tile_position=(0, base),
                        skip_group_check=True,
                    )
                    nc.tensor.matmul(
                        p_ps[base : base + 1, 512:D], lhsT=u, rhs=xs[:, 512:D],
                        start=(i < 4), stop=(i >= NT - 4),
                        tile_position=(0, base),
                        skip_group_check=True,
                    )

            for i in range(NT + 3):
                if i < NT:
                    emit_front(i)
                if i == 1:
                    emit_params()
                if 1 <= i <= NT:
                    emit_back(i - 1)
                k = i - 3
                if k >= 3 and k % 4 == 3:
                    emit_pool_group(k)

            # write out unnormalized p rows and the Z partials; the host sums
            # rows 0/32/64/96 of out_p and all of out_z
            p_sb = singles.tile([P, D], F32)
            for base in (0, 32, 64, 96):
                nc.scalar.copy(
                    out=p_sb[base : base + 1, :], in_=p_ps[base : base + 1, :]
                )
            nc.sync.dma_start(out=p_d[:, :], in_=p_sb[0:97:32, :])
            nc.sync.dma_start(out=z_d[:, :], in_=zg)

    if split_waits:
        _split_excess_waits(nc)
    return nc


def _split_excess_waits(nc: bass.Bass) -> None:
    """Walrus accepts a single HW sync-wait per instruction (EventSemaphore
    excepted). Tile can attach more (data dep + DMA-lane reuse). Move all but
    one wait onto InstEventSemaphore(s) inserted just before, on the same
    engine — the sequencer executes waits in order, so semantics are
    unchanged."""
    fn = nc.m.functions[0]
    for blk in fn.blocks:
        insts = blk.instructions
        new_insts = []
        for inst in insts:
            si = inst.sync_info
            if (
                not isinstance(inst, mybir.InstEventSemaphore)
                and si is not None
                and len(si.on_wait) > 1
            ):
                waits = list(si.on_wait)
                for w in waits[:-1]:
                    ev = mybir.InstEventSemaphore(
                        name=nc.get_next_instruction_name(), ins=[], outs=[]
                    )
                    ev.engine = inst.engine
                    ev.sync_info = mybir.SyncInfo(on_wait=[w], on_update=[])
                    new_insts.append(ev)
                inst.sync_info = mybir.SyncInfo(
                    on_wait=waits[-1:], on_update=list(si.on_update)
                )
            new_insts.append(inst)
        blk.instructions = new_insts


_CACHE: dict = {}
LAST_RESULT = None


def _get_nc() -> bass.Bass:
    if "nc" not in _CACHE:
        _CACHE["nc"] = _build()
    return _CACHE["nc"]


def kernel(x: np.ndarray, att_v: np.ndarray, att_W: np.ndarray) -> np.ndarray:
    global LAST_RESULT
    assert x.shape == (NCORES, S, D), x.shape
    nc = _get_nc()
    in_maps = [
        {
            "x": np.ascontiguousarray(x[b], dtype=np.float32),
            "att_v": np.ascontiguousarray(att_v, dtype=np.float32),
            "att_W": np.ascontiguousarray(att_W, dtype=np.float32),
        }
        for b in range(NCORES)
    ]
    res = run_bass_kernel_spmd(nc, in_maps, core_ids=list(range(NCORES)))
    LAST_RESULT = res
    outs = []
    for b in range(NCORES):
        p = res.results[b]["out_p"].sum(axis=0, dtype=np.float64)
        z = res.results[b]["out_z"].sum(dtype=np.float64)
        outs.append(p / z)
    return np.stack(outs).astype(np.float32)
